# revision 9
# baseline (speedup 1.0000x reference)
"""Causal multi-head attention on 8 Trainium2 NeuronCores.

Problem: Q,K,V [2,16,2048,128] f32, out = causal-softmax(QK^T/sqrt(128)) V.
Sharding: batch*heads = 32 slices -> 4 heads per core across 8 cores; each
core computes its heads fully independently (no collectives).

Per-head pipeline on one core (S=2048, D=128):
  - Host pre-transposes Q,K per head to [128(d), 2048(seq)] bf16 and V to
    [128(p), 16(t), 128(d)] bf16; input DMAs are chunked per 512 columns so
    compute starts as soon as the first q-block's operands land.
  - Scores transposed, one k-tile strip at a time: S^T[k,q] = kt_tile.T @
    qt_block into PSUM [128, 512] (1 bank; 4 strips pipelined).
  - Causal mask added only on the 128x128 diagonal subtile by a bf16 matmul
    diag(-1e9) @ tri01 (fully-masked subtiles are skipped everywhere).
  - exp alternates between two engines (per-strip): ACT computes exact exp
    with fused scale (bf16 out); DVE computes the Schraudolph bit-trick
    (one fused tensor_scalar: i16 = s*A + B, bitcast bf16 == 2^(i/128),
    ~3% per-weight error, fine at rel tol 2e-2). HW-verified: the DVE
    f32->i16 convert rounds RNE and saturates, so -1e9-masked scores become
    -32768 = 0x8000 = bf16 -0.0 -> weight exactly 0. Diagonal strips skip
    the leading fully-masked subtiles (~15% fewer exp columns).
  - PV in [q,d] layout: for each 128-query subtile, out[q, 0:129] +=
    W^T[:, qsub].T @ [V|1] (bf16, N=129), accumulated over k-tiles in PSUM;
    column 128 accumulates the softmax denominators.
  - No on-chip normalize: the [q, 0:129] PSUM tiles are copied to SBUF f16
    (ACT/DVE alternating) and DMA'd out; the host divides by column 128.
"""

import sys

sys.path.insert(0, "/opt/trn_rl_repo")

from contextlib import ExitStack

import numpy as np
import ml_dtypes

import concourse.bass as bass
import concourse.bacc as bacc
import concourse.mybir as mybir
import concourse.tile as tile

F32 = mybir.dt.float32
BF16 = mybir.dt.bfloat16
F16 = mybir.dt.float16
I16 = mybir.dt.int16

B, H, S, D = 2, 16, 2048, 128
NCORES = 8
HPC = (B * H) // NCORES  # 4 heads per core
P = 128                  # partition dim / k-tile / q-subtile size
QB = 512                 # q block width (scores moving free dim)
NQB = S // QB            # 4
NKT = S // P             # 16 k-tiles per head
VW = 132                 # padded [V|1] row width (129 used)
OW = 2 * (P + 1)         # packed output bank width: two [q,129] subtiles
SCALE = 1.0 / float(np.sqrt(128.0))
NEG = -1.0e9

# Schraudolph exp on DVE: exp(s*SCALE) ~= bitcast_bf16(i16(s*SCH_MUL + SCH_ADD))
SCH_A = 128.0 / float(np.log(2.0))       # 2^7 / ln 2
SCH_C = 5.5                              # calibrated for RNE convert
SCH_MUL = SCH_A * SCALE
SCH_ADD = 127.0 * 128.0 - SCH_C

Exp = mybir.ActivationFunctionType.Exp
ALU_MULT = mybir.AluOpType.mult
ALU_ADD = mybir.AluOpType.add

# cost-model ns for a c-column strip on each engine (for greedy balancing)
def _cost_act(c):
    return 0.833 * c + 143.0


def _cost_dve(c):
    return 1.042 * c + 126.0


def _exp_assignment():
    """Greedy per-head assignment of exp strips to ACT ('A') / DVE ('D'),
    balancing engine time. Returns {(qb, kt): 'A'|'D'}."""
    acc_a = 4 * 358.0   # ACT's share of the po->SBUF copies per head
    acc_d = 4 * 394.0
    assign = {}
    for qb in range(NQB):
        for kt in range(4 * (qb + 1)):
            r = kt - 4 * qb
            ncols = QB - max(r, 0) * P
            ca, cd = _cost_act(ncols), _cost_dve(ncols)
            if acc_a + ca <= acc_d + cd:
                assign[(qb, kt)] = "A"
                acc_a += ca
            else:
                assign[(qb, kt)] = "D"
                acc_d += cd
    return assign


EXP_ASSIGN = _exp_assignment()


def _emit_core(tc: tile.TileContext, ctx: ExitStack, qt_in, kt_in, v_in, o_out,
               diag_in, tri_in):
    nc = tc.nc

    const = ctx.enter_context(tc.tile_pool(name="const", bufs=1))
    big = ctx.enter_context(tc.tile_pool(name="big", bufs=2))
    wpool = ctx.enter_context(tc.tile_pool(name="w", bufs=6))
    ps_s = ctx.enter_context(tc.tile_pool(name="ps_s", bufs=4, space=bass.MemorySpace.PSUM))
    ps_o = ctx.enter_context(tc.tile_pool(name="ps_o", bufs=4, space=bass.MemorySpace.PSUM))

    diagneg = const.tile([P, P], BF16, tag="diagneg")
    trid = const.tile([P, P], BF16, tag="trid")
    zerostat = const.tile([P, P], BF16, tag="zerostat")
    nc.sync.dma_start(diagneg[:], diag_in)
    nc.sync.dma_start(trid[:], tri_in)
    nc.gpsimd.memset(zerostat[:], 0.0)

    for h in range(HPC):
        # ---- load this head's tensors, chunked per q-block / 4-k-tiles ----
        qt = big.tile([P, S], BF16, tag="qt")
        kt = big.tile([P, S], BF16, tag="kt")
        vb = big.tile([P, NKT, VW], BF16, tag="vb")
        for c in range(NQB):
            cs = slice(c * QB, (c + 1) * QB)
            nc.sync.dma_start(kt[:, cs], kt_in[h][:, cs])
            nc.sync.dma_start(qt[:, cs], qt_in[h][:, cs])
            nc.sync.dma_start(
                vb[:, 4 * c:4 * c + 4, 0:P],
                v_in[h][:, cs].rearrange("p (t d) -> p t d", t=4))
        nc.gpsimd.memset(vb[:, :, P:P + 1], 1.0)

        obuf = big.tile([P, NQB, 2 * OW], F16, tag="obuf")

        for qb in range(NQB):
            nkt = 4 * (qb + 1)  # causal: k-tiles 0..nkt-1
            po = []
            for _b in range(2):
                po_t = ps_o.tile([P, OW], F32, tag="po")
                po.append(po_t)
                # start=True clears has_written for the WHOLE bank, so each
                # bank gets exactly one start: a zero-fill matmul claiming
                # both packed accumulation groups; all PV matmuls accumulate.
                nc.tensor.matmul(po_t[:], zerostat[:], kt[:, 0:OW],
                                 start=True, stop=False)

            def po_ap(j):
                return po[j // 2][:, (j % 2) * (P + 1):(j % 2) * (P + 1) + P + 1]

            for kkt in range(nkt):
                r = kkt - 4 * qb
                j0 = max(r, 0)
                ps = ps_s.tile([P, QB], F32, tag="ps")
                nc.tensor.matmul(ps[:, j0 * P:QB],
                                 kt[:, kkt * P:(kkt + 1) * P],
                                 qt[:, qb * QB + j0 * P:(qb + 1) * QB],
                                 start=True, stop=(r < 0))
                if r >= 0:  # mask only the 128-wide diagonal subtile
                    nc.tensor.matmul(ps[:, r * P:(r + 1) * P], diagneg[:],
                                     trid[:], start=False, stop=True)
                # exp on the valid region only
                wi = wpool.tile([P, QB], I16, tag="w")
                if EXP_ASSIGN[(qb, kkt)] == "A":
                    nc.scalar.activation(wi[:, j0 * P:QB].bitcast(BF16),
                                         ps[:, j0 * P:QB], Exp, scale=SCALE)
                else:
                    nc.vector.tensor_scalar(wi[:, j0 * P:QB], ps[:, j0 * P:QB],
                                            SCH_MUL, SCH_ADD, ALU_MULT, ALU_ADD)
                wap = wi[:].bitcast(BF16)
                # PV accumulation
                for j in range(j0, 4):
                    nc.tensor.matmul(po_ap(j),
                                     wap[:, j * P:(j + 1) * P],
                                     vb[:, kkt, 0:P + 1],
                                     start=False, stop=(kkt == 4 * qb + j))

            # ---- copy the two packed output banks to SBUF (f16), DMA out ----
            nc.scalar.copy(obuf[:, qb, 0:OW], po[0][:])
            nc.vector.tensor_copy(obuf[:, qb, OW:2 * OW], po[1][:])
            nc.sync.dma_start(o_out[h][:, qb * 2 * OW:(qb + 1) * 2 * OW],
                              obuf[:, qb, :])


def build_nc(runs=1):
    nc = bacc.Bacc("TRN2", target_bir_lowering=False, debug=False)
    qt = nc.dram_tensor("qt", [HPC, P, S], BF16, kind="ExternalInput")
    kt = nc.dram_tensor("kt", [HPC, P, S], BF16, kind="ExternalInput")
    v = nc.dram_tensor("v", [HPC, P, S], BF16, kind="ExternalInput")
    diag = nc.dram_tensor("diagneg", [P, P], BF16, kind="ExternalInput")
    tri = nc.dram_tensor("trid", [P, P], BF16, kind="ExternalInput")
    o = nc.dram_tensor("o", [HPC, P, NQB * 2 * OW], F16, kind="ExternalOutput")
    with tile.TileContext(nc) as tc:
        with ExitStack() as ctx:
            if runs > 1:
                with tc.For_i(0, runs, 1):
                    _emit_core(tc, ctx, qt.ap(), kt.ap(), v.ap(), o.ap(),
                               diag.ap(), tri.ap())
            else:
                _emit_core(tc, ctx, qt.ap(), kt.ap(), v.ap(), o.ap(),
                           diag.ap(), tri.ap())
    nc.compile()
    return nc


def make_consts():
    diag = (NEG * np.eye(P)).astype(ml_dtypes.bfloat16)
    # trid[c, q] = 1 where in-tile key index c > query index q (masked)
    c = np.arange(P)[:, None]
    q = np.arange(P)[None, :]
    trid = (c > q).astype(ml_dtypes.bfloat16)
    return diag, trid


def make_in_maps(Q, K, V):
    diag, trid = make_consts()
    bf = ml_dtypes.bfloat16
    Qr = np.asarray(Q, dtype=np.float32).reshape(B * H, S, D)
    Kr = np.asarray(K, dtype=np.float32).reshape(B * H, S, D)
    Vr = np.asarray(V, dtype=np.float32).reshape(B * H, S, D)
    QT = np.ascontiguousarray(Qr.transpose(0, 2, 1)).astype(bf)  # [32, 128, 2048]
    KT = np.ascontiguousarray(Kr.transpose(0, 2, 1)).astype(bf)
    # V -> [head, p, t*128 + d] with V[head, t*128 + p, d]
    VT = np.ascontiguousarray(
        Vr.reshape(B * H, NKT, P, D).transpose(0, 2, 1, 3).reshape(B * H, P, S)
    ).astype(bf)
    in_maps = []
    for c in range(NCORES):
        sl = slice(c * HPC, (c + 1) * HPC)
        in_maps.append({
            "qt": QT[sl], "kt": KT[sl], "v": VT[sl],
            "diagneg": diag, "trid": trid,
        })
    return in_maps


_NC = None


def kernel(Q: np.ndarray, K: np.ndarray, V: np.ndarray) -> np.ndarray:
    from concourse.bass_utils import run_bass_kernel_spmd

    global _NC
    if _NC is None:
        _NC = build_nc()
    nc = _NC

    in_maps = make_in_maps(Q, K, V)
    res = run_bass_kernel_spmd(nc, in_maps, core_ids=list(range(NCORES)))
    out = np.concatenate([res.results[c]["o"] for c in range(NCORES)], axis=0)
    # o[h, p, qb*516 + slot*129 + c], q = qb*512 + slot*128 + p
    out = out.astype(np.float32).reshape(B * H, P, NQB, 4, P + 1)
    num = out[..., 0:P]           # [32, p, qb, slot, d]
    den = out[..., P]             # [32, p, qb, slot]
    o = num / den[..., None]
    o = o.transpose(0, 2, 3, 1, 4)  # [32, qb, slot, p, d]
    return np.ascontiguousarray(o.reshape(B, H, S, D))
